# revision 16
# baseline (speedup 1.0000x reference)
"""Cross-attention kernel for Trainium2 (Bass/Tile), 8-core SPMD. v2.

Problem: single-head cross attention over flattened 64x64 spatial positions.
  Q = Wq @ x_q                 [B,128,4096]   (bq = 0)
  K = Wk @ x_kv                [B,128,4096]   (bk = 0)
  V = Wv @ x_kv + bv           [B,128,4096]
  attn = softmax(0.25 * Q^T K) over keys      [B,4096,4096]
  out  = Wo @ (attn @ V^T)^T + bo + x_q       [B,128,64,64]

Sharding: data-parallel over batch (4 samples) x 2-way query split = 8 cores.
Each core: 2048 queries vs all 4096 keys of one sample.

Host-side algebraic folds (all exact for this problem's zero q/k biases):
  - Wq folded into the K projection:  S = x_q^T (SCALE Wq^T Wk) x_kv.
    Removes the Q projection matmuls AND the Q PSUM->SBUF copies; the
    S-matmul moving operand is the raw bf16 x_q loaded from HBM.
    (bq = 0 makes the per-key bias term vanish; the per-query terms are
    softmax-invariant regardless.)
  - Wo folded into Wv:  out = attn @ (Wo Wv x_kv)^T + (Wo bv + bo) + x_q,
    using sum_k attn[q,k] = 1. Removes the output projection matmul AND
    gives the PV matmul output directly in [channel, position] layout.
  - (Wo bv + bo) folded into the f32 residual input.

Device pipeline per core:
  setup: K'[c,k]  = wk2.T @ x_kv   fp8 DoubleRow (256-deep contraction),
                    ACT copy -> bf16 SBUF (x 1/WS_K)
         VT[k,o]  = x_kv_chunk.T @ wv2  fp8 DR -> fp8 SBUF (keeps x WS_V)
  per q-tile (1024 queries), per k-chunk (128 keys):
         S^T_chunk[k,q] = K'_chunk.T @ xq16_tile   (PE, bf16 -> PSUM)
         P_chunk = exp(S^T_chunk)                  (ACT table-exp or DVE
                    Schraudolph fast-exp, PSUM -> SBUF fp8, ~3:2 split)
         outT   += VT_chunk.T @ P_chunk            (PE fp8 DR accumulate)
         sum    += ones.T @ P_chunk                (PE fp8 DR accumulate)
  tail:  r = recip(sum) (DVE f32); bc = oner.T @ r (PE, f32r moving);
         outf = outT * bc (DVE); outf += xqres (GPSIMD); DMA out (f32).

DMA: sync HWDGE ring carries x_kv (4 chunks, both r-halves per trigger) so
K'-projection starts ~2.5us in; gpsimd SWDGE ring carries weights, xq16 and
(last) the residual, which is only needed at the tail.

No max-subtraction in softmax: |0.25*Q^T K| <= ~1.3 for this problem's fixed
input distribution (weights scaled by 0.02), so exp never overflows and
softmax(x) == exp(x)/sum(exp(x)) exactly.
"""

import sys

if "/opt/trn_rl_repo" not in sys.path:
    sys.path.insert(0, "/opt/trn_rl_repo")

import numpy as np
import ml_dtypes

B, CQ, CKV, H, W = 4, 128, 256, 64, 64
N = H * W            # 4096 positions
NH = N // 2          # 2048 queries per core
QT = 1024            # query tile (free-dim of the S^T matmuls)
NQT = NH // QT       # 2 query tiles per core
KC = 128             # key chunk (partition dim of S^T)
NKC = N // KC        # 32 key chunks
SCALE = (CQ // 8) ** (-0.5)  # 0.25

# --- engine load-balancing knobs ---
# exp engine per k-chunk: ACT (exact spline exp) vs DVE (Schraudolph fast-exp:
# uint8 = A8*x + B8 is the fp8-e4m3 bit pattern of e^x, one tensor_scalar op).
# strict within-pair alternation (even chunk -> ACT, odd -> DVE) so the two
# chunks of every pair run concurrently; a few "relief" pairs go both-ACT to
# offset the DVE's recip/normalize tail work (global pair index 0..31).
RELIEF_PAIRS = frozenset({10, 20, 26})
def EXP_DVE(gp, kc):
    return (kc % 2 == 1) and (gp not in RELIEF_PAIRS)

# fp8 e4m3 Schraudolph: uint8 = A8*x + B8 is the e4m3 bit pattern of e^x
# (max rel err ~7%, cancelled by softmax renormalization)
SCHRAUD_A8 = 8.0 / np.log(2.0)
SCHRAUD_B8 = 55.62
# fp8 weight scales to keep quantized values in e4m3 normal range
WS_K = 256.0         # folded SCALE*Wq^T*Wk entries ~1e-3
WS_V = 64.0          # folded Wo*Wv entries ~5e-3

_cache = {}


def _build_program():
    import concourse.bass as bass  # noqa: F401
    from concourse import bacc
    import concourse.mybir as mybir
    import concourse.tile as tile

    f32 = mybir.dt.float32
    f32r = mybir.dt.float32r
    bf16 = mybir.dt.bfloat16
    fp8 = mybir.dt.float8e4
    u8 = mybir.dt.uint8
    AF = mybir.ActivationFunctionType
    ALU = mybir.AluOpType

    nc = bacc.Bacc(
        "TRN2",
        target_bir_lowering=False,
        debug=False,
        enable_asserts=False,
        num_devices=8,
    )

    # ---- DRAM I/O (per-core shapes) ----
    # wpack: cols 0:256 = wk2 (r-major pairs), cols 256:512 = wv2
    d_wpack = nc.dram_tensor("wpack", [128, 512], fp8, kind="ExternalInput").ap()
    d_xq16 = nc.dram_tensor("xq16", [CQ, NH], bf16, kind="ExternalInput").ap()
    d_xqres = nc.dram_tensor("xqres", [CQ, NH], f32, kind="ExternalInput").ap()
    # xkv fp8, layout [c' within half (partition), (r-half, n)]
    d_xkv8 = nc.dram_tensor("xkv8", [128, 2 * N], fp8, kind="ExternalInput").ap()
    d_out = nc.dram_tensor("out", [CQ, NH], f32, kind="ExternalOutput").ap()

    DR = mybir.MatmulPerfMode.DoubleRow

    with tile.TileContext(nc) as tc:
        with (
            tc.tile_pool(name="const", bufs=1) as cp,
            tc.tile_pool(name="big", bufs=1) as bp,
            tc.tile_pool(name="pt", bufs=4) as ptp,
            tc.tile_pool(name="misc", bufs=2) as mp,
            tc.tile_pool(name="outp", bufs=3) as op_,
            tc.tile_pool(name="mm", bufs=2, space="PSUM") as mm,
            tc.tile_pool(name="sump", bufs=1, space="PSUM") as sump,
            tc.tile_pool(name="pv", bufs=1, space="PSUM") as pvp,
        ):
            # ---- input DMAs. sync HWDGE ring: xkv8 chunks (K'/VT are the
            # first consumers). gpsimd SWDGE ring: weights + xq16, then the
            # residual last (only needed at the tail). ----
            # weights + xq16 ride the ACT HWDGE ring (parallel to the sync
            # SP ring carrying xkv8); the tail-only residual rides gpsimd
            wpack = cp.tile([128, 512], fp8, name="wpack")
            nc.scalar.dma_start(wpack, d_wpack)
            # xkv8 host layout: [p, (G, r, n)] at 512-key group granularity —
            # each group's two r-halves are column-adjacent. First two DMA
            # chunks are single groups so the K' projection starts early.
            xkv8 = cp.tile([128, 2 * N], fp8, name="xkv8")
            for lo, hi in ((0, 1), (1, 2), (2, 4), (4, 6), (6, 8)):
                sl = slice(lo * 1024, hi * 1024)
                nc.sync.dma_start(xkv8[:, sl], d_xkv8[:, sl])
            xq16 = cp.tile([128, NH], bf16, name="xq16")
            nc.scalar.dma_start(xq16, d_xq16)
            xqres = cp.tile([128, NH], f32, name="xqres")
            nc.gpsimd.dma_start(xqres, d_xqres)

            # pair-ones for the DoubleRow softmax-sum matmuls; 16-col halves
            # because the DR weight AP needs pair-step % 16 == 0
            ones8 = cp.tile([128, 32], fp8, name="ones8")
            nc.gpsimd.memset(ones8, 1.0)
            # broadcast-ones row carries the 1/WS_V compensation for the
            # scaled V' weights; f32r so the moving recip stays full-precision
            oner = cp.tile([1, 128], f32, name="oner")
            nc.gpsimd.memset(oner, 1.0 / WS_V)

            # DoubleRow operand views: 4D [p, r, 1, n] so the pair dim lands
            # in the ISA pattern's num_elem[2] slot (outermost, count 2)
            wk3 = wpack[:, 0:256].rearrange("p (r one m) -> p r one m", r=2, one=1)
            wv3 = wpack[:, 256:512].rearrange("p (r one m) -> p r one m", r=2, one=1)
            ones3 = ones8.rearrange("p (r one m) -> p r one m", r=2, one=1)[
                :, :, :, 0:1
            ]

            Ksb = bp.tile([128, N], bf16)
            VTsb = bp.tile([128, N], fp8)

            # ---- setup, per 1024-key chunk of xkv (tracks the DMA):
            # K' = wk2.T @ xkv (fp8 DR), ACT copy -> bf16 (undo WS_K)
            # VT[k,o] = xkv_chunk.T @ wv2 (fp8 DR), DVE copy -> fp8 (keep WS_V)
            # per-512-key-group pair views: [p, r(stride 512), 1, n(512)]
            xkvG = [
                xkv8[:, G * 1024:(G + 1) * 1024].rearrange(
                    "p (r one n) -> p r one n", r=2, one=1
                )
                for G in range(8)
            ]
            for g in range(4):
                kp_ps = mm.tile([128, 1024], f32, tag="mm", name="kp_ps")
                for j in range(2):
                    nc.tensor.matmul(
                        kp_ps[:, j * 512:(j + 1) * 512], wk3,
                        xkvG[g * 2 + j], start=True, stop=True, perf_mode=DR,
                    )
                nc.scalar.activation(
                    Ksb[:, g * 1024:(g + 1) * 1024], kp_ps, AF.Identity,
                    scale=1.0 / WS_K,
                )
                for h in range(2):
                    G = g * 2 + h
                    vt_ps = mm.tile([128, 512], f32, tag="mm", name="vt_ps")
                    for j in range(4):
                        nc.tensor.matmul(
                            vt_ps[:, j * 128:(j + 1) * 128],
                            xkvG[G][:, :, :, j * KC:(j + 1) * KC], wv3,
                            start=True, stop=True, perf_mode=DR,
                        )
                    nc.vector.tensor_copy(VTsb[:, G * 512:(G + 1) * 512], vt_ps)

            # ---- main attention loop (software-pipelined at pair level:
            # S-matmuls + exp of pair p+LEAD are emitted before the PV/sum
            # DoubleRow matmuls of pair p, so the PE never head-of-line
            # blocks on the exp handoff) ----
            NPAIR = NKC // 2
            LEAD = 2
            for qt in range(NQT):
                qsl0 = qt * QT
                pv_ps = pvp.tile([128, QT], f32, tag="pv", name="pv_ps")
                sum_ps = sump.tile([1, QT], f32, tag="sum", name="sum_ps")
                pts = {}
                for step in range(NPAIR + LEAD):
                    if step < NPAIR:
                        pt2 = ptp.tile([128, 2 * QT], fp8, tag="pt", name="pt2")
                        pts[step] = pt2
                        for kc in (2 * step, 2 * step + 1):
                            ksl = slice(kc * KC, (kc + 1) * KC)
                            s_ps = mm.tile([128, QT], f32, tag="mm", name="s_ps")
                            for j in range(QT // 512):
                                nc.tensor.matmul(
                                    s_ps[:, j * 512:(j + 1) * 512],
                                    Ksb[:, ksl],
                                    xq16[:, qsl0 + j * 512: qsl0 + (j + 1) * 512],
                                    start=True,
                                    stop=True,
                                )
                            half = slice((kc % 2) * QT, (kc % 2) * QT + QT)
                            if EXP_DVE(qt * NPAIR + step, kc):
                                nc.vector.tensor_scalar(
                                    pt2[:, half].bitcast(u8), s_ps,
                                    SCHRAUD_A8, SCHRAUD_B8,
                                    op0=ALU.mult, op1=ALU.add,
                                )
                            else:
                                nc.scalar.activation(pt2[:, half], s_ps, AF.Exp)
                    if step >= LEAD:
                        p = step - LEAD
                        pt3 = pts.pop(p).rearrange(
                            "q (r one n) -> q r one n", r=2, one=1
                        )
                        vt3 = VTsb[:, p * 256:(p + 1) * 256].rearrange(
                            "q (r one m) -> q r one m", r=2, one=1
                        )
                        for j in range(QT // 512):
                            jsl = slice(j * 512, (j + 1) * 512)
                            nc.tensor.matmul(
                                pv_ps[:, jsl], vt3, pt3[:, :, :, jsl],
                                start=(p == 0), stop=(p == NPAIR - 1),
                                perf_mode=DR,
                            )
                            nc.tensor.matmul(
                                sum_ps[:, jsl], ones3, pt3[:, :, :, jsl],
                                start=(p == 0), stop=(p == NPAIR - 1),
                                perf_mode=DR,
                            )
                # tail, pipelined per 512-block across DVE/PE/DVE/GPSIMD/DMA:
                # recip -> f32r broadcast matmul -> normalize -> residual ->
                # store. The residual add runs on GPSIMD (SBUF-only) so the
                # DVE is free for the next tile's exps.
                recip = mp.tile([1, QT], f32, name="recip")
                for j in range(QT // 512):
                    jsl = slice(j * 512, (j + 1) * 512)
                    osl = slice(qsl0 + j * 512, qsl0 + (j + 1) * 512)
                    nc.vector.reciprocal_approx_fast(
                        recip[:, jsl], sum_ps[:, jsl]
                    )
                    bc_ps = mm.tile([128, 512], f32, tag="mm", name="bc_ps")
                    nc.tensor.matmul(
                        bc_ps, oner, recip[:, jsl], start=True, stop=True,
                    )
                    # DVE can only read one PSUM operand per instruction:
                    # stage the broadcast through SBUF on the scalar engine
                    bc_sb = mp.tile([128, 512], f32, name="bc_sb")
                    nc.scalar.copy(bc_sb, bc_ps)
                    outf = op_.tile([128, 512], f32, name="outf")
                    nc.vector.tensor_mul(outf, pv_ps[:, jsl], bc_sb)
                    nc.vector.tensor_add(outf, outf, xqres[:, osl])
                    eng = nc.sync if j % 2 == 0 else nc.gpsimd
                    eng.dma_start(d_out[:, osl], outf)

    nc.compile()
    return nc


def _get_program():
    if "nc" not in _cache:
        _cache["nc"] = _build_program()
    return _cache["nc"]


def _make_in_maps(x_q, x_kv, Wq, bq, Wk, bk, Wv, bv, Wo, bo):
    bf16 = ml_dtypes.bfloat16
    f32 = np.float32
    fp8 = ml_dtypes.float8_e4m3fn

    x_q = np.asarray(x_q, dtype=f32).reshape(B, CQ, N)
    x_kv = np.asarray(x_kv, dtype=f32).reshape(B, CKV, N)
    Wq = np.asarray(Wq, dtype=f32)
    Wk = np.asarray(Wk, dtype=f32)
    Wv = np.asarray(Wv, dtype=f32)
    Wo = np.asarray(Wo, dtype=f32)
    bq = np.asarray(bq, dtype=f32)
    bk = np.asarray(bk, dtype=f32)
    bv = np.asarray(bv, dtype=f32)
    bo = np.asarray(bo, dtype=f32)

    # the Wq fold drops the per-key bias term bq^T Wk x_kv; only valid when
    # bq == 0 (true for this problem). bk only contributes softmax-invariant
    # per-query terms and drops for any bk.
    assert np.all(bq == 0.0), "Wq fold requires bq == 0"

    # host-side algebraic folds
    W2 = (Wq.T @ Wk) * SCALE           # [128, 256]
    Wv2 = Wo @ Wv                      # [128, 256]
    b_final = Wo @ bv + bo             # [128]
    w2T = W2.T * WS_K                  # [256, 128], scaled for fp8 range
    wvT = Wv2.T * WS_V                 # [256, 128], scaled for fp8 range
    # r-major pair layout for DoubleRow: [c' within half, (half, col)]
    wk8 = np.stack([w2T[:128], w2T[128:]], axis=1).reshape(128, 256)
    wv8 = np.stack([wvT[:128], wvT[128:]], axis=1).reshape(128, 256)
    wpack = np.concatenate([wk8, wv8], axis=1).astype(fp8)  # [128, 512]

    in_maps = []
    for core in range(8):
        b, half = divmod(core, 2)
        sl = slice(half * NH, (half + 1) * NH)
        # [p, (G, r, n)]: 512-key group G holds both c'-halves adjacently
        xkv8 = (
            x_kv[b].reshape(2, 128, 8, 512).transpose(1, 2, 0, 3)
            .reshape(128, 2 * N)
        )
        in_maps.append(
            {
                "xq16": x_q[b][:, sl].astype(bf16),
                "xqres": np.ascontiguousarray(
                    x_q[b][:, sl] + b_final[:, None]
                ),
                "xkv8": xkv8.astype(fp8),
                "wpack": np.ascontiguousarray(wpack),
            }
        )
    return in_maps


def _assemble(results):
    out = np.empty((B, CQ, N), dtype=np.float32)
    for core in range(8):
        b, half = divmod(core, 2)
        out[b][:, half * NH:(half + 1) * NH] = results[core]["out"]
    return out.reshape(B, CQ, H, W)


def run_raw(in_maps, trace=False, core_ids_override=None, **kwargs):
    from concourse.bass_utils import run_bass_kernel_spmd

    nc = _get_program()
    core_ids = core_ids_override or list(range(8))
    return run_bass_kernel_spmd(
        nc, in_maps, core_ids=core_ids, trace=trace, **kwargs
    )


def kernel(**inputs) -> np.ndarray:
    in_maps = _make_in_maps(**inputs)
    res = run_raw(in_maps)
    return _assemble(res.results)


def kernel_profiled(**inputs):
    """Returns (output, BassKernelResults-with-trace)."""
    in_maps = _make_in_maps(**inputs)
    res = run_raw(in_maps, trace=True)
    return _assemble(res.results), res
